# revision 27
# baseline (speedup 1.0000x reference)
"""Multi-head attention (B=2, L=2048, DIM=1024, H=16) on 8 TRN2 NeuronCores.

Sharding: core c = (batch b = c//4, head-group hg = c%4 of 4 heads / 256 dims).
Data parallel over B, tensor parallel over heads; Q/K/V weights column-sharded.
Each core is fully independent (no collectives); host gathers the 8 output
shards.

Per-core layout trick: everything is computed transposed (seq on the free
axis) so no on-device transposes are needed:
  QT/KT [hd, seq]  <- matmul(lhsT=W_slice, rhs=xT)       (xT transposed on host)
  ST    [k, q]     <- matmul(lhsT=KT_head, rhs=QT_head)  (= scores transposed)
  E     = exp(ST)         (max-subtraction skipped: logits are N(0,1)-scaled,
                           mask only subtracts -> exp stays in [e^-65, e^5])
  Emask = E * exp(-60*mask)^T                            (mask exp'd on host)
  OT    [hd+ones, q] <- matmul(lhsT=[V | ones], rhs=Emask) accumulated over k;
                        rows 64..127 give the softmax denominator replicated,
                        so out = OT[0:64] / OT[64:128] needs no partition
                        broadcast.
The 1/sqrt(64) score scale is folded into Wq on the host.
Biases are zeros per the problem spec and are skipped.

Pipeline structure (v2): the kernel is ACT(exp)-throughput-bound, so the
instruction streams are laid out to keep ScalarE fed from ~first-DMA-landing
to the end and to keep TensorE saturated (its DVFS p-state only reaches
2.4 GHz when continuously busy):
  - All QK/V projections are emitted just-in-time, interleaved into the
    attention iteration stream instead of as a serial head phase.
  - The mask multiply is one [128,1024] tensor_tensor per (head-pair, kb)
    using a stride-0 broadcast AP on the [128,512] mask tile (covers both
    heads in one DVE op).
  - The softmax reciprocal runs on DVE (reciprocal_approx_fast, ~51 ULP)
    instead of Ln+Exp on ScalarE.
  - The PV ones-rows are memset on GpSimd (free engine at startup).
"""

import sys

for _p in ("/opt/trn_rl_repo",):
    if _p not in sys.path:
        sys.path.append(_p)

import numpy as np
import ml_dtypes

import concourse.tile as tile
from concourse import bacc, mybir
from concourse.bass_utils import run_bass_kernel_spmd

BF16 = ml_dtypes.bfloat16

B, L, DIM, H = 2, 2048, 1024, 16
HPC = 4          # heads per core
HD = DIM // H    # 64
GW = HPC * HD    # 256, head-group width per core
N_CORES = 8
MASK_SCALE = -60.0
SCALE = float(HD) ** -0.5

P = 128
KD = DIM // P        # 8  contraction blocks for projections
NSEQ = L // P        # 16 seq blocks (k blocks)
QP = 512             # q panel width
NQP = L // QP        # 4 q panels
NITER = NQP * 2 * NSEQ  # 128 attention iterations (j, hp, kb)

_CACHE = {}


def _build_nc():
    f32 = mybir.dt.float32
    bf16 = mybir.dt.bfloat16

    nc = bacc.Bacc("TRN2", target_bir_lowering=False)

    xT = nc.declare_dram_parameter("xT", [DIM, L], bf16, isOutput=False)
    expmT = nc.declare_dram_parameter("expmT", [L, L], bf16, isOutput=False)
    wq = nc.declare_dram_parameter("wq", [DIM, GW], bf16, isOutput=False)
    wk = nc.declare_dram_parameter("wk", [DIM, GW], bf16, isOutput=False)
    wv = nc.declare_dram_parameter("wv", [DIM, GW], bf16, isOutput=False)
    outT = nc.declare_dram_parameter("outT", [GW, L], f32, isOutput=True)

    with tile.TileContext(nc) as tc:
        with (
            tc.tile_pool(name="persist", bufs=1) as persist,
            tc.tile_pool(name="e", bufs=6) as e_pool,
            tc.tile_pool(name="eh", bufs=6) as eh_pool,
            tc.tile_pool(name="osb", bufs=2) as osb_pool,
            tc.tile_pool(name="res", bufs=2) as res_pool,
            tc.tile_pool(name="ps_proj", bufs=2, space="PSUM") as ps_proj,
            tc.tile_pool(name="ps_s", bufs=2, space="PSUM") as ps_s,
            tc.tile_pool(name="ps_o", bufs=1, space="PSUM") as ps_o,
        ):
            # ---- input DMA ----
            # Every dma_start costs ~600ns of serial issue time on its DGE
            # queue (SP), so the DMA plan minimizes instruction count and
            # splits issue across the two HWDGE engines (SP + ACT):
            #   SP:  xt halves 0 + wk (the k00 deps), mask rows 0-3,
            #        xt halves 1, mask rows 4-15
            #   ACT: wq, wv (ACT is idle until the first EXP anyway)
            # The mask is loaded once as 16 persistent whole-row tiles and
            # stays resident for all four q panels.
            HC = L // 2  # xt column-half width
            xt_sb = [[None] * 2 for _ in range(KD)]
            w_sb = {"q": [None] * KD, "k": [None] * KD, "v": [None] * KD}

            def load_w(name, dram, kd, eng):
                w = persist.tile([P, GW], bf16, tag=f"w{name}{kd}", name=f"w{name}{kd}")
                eng.dma_start(w[:], dram[kd * P : (kd + 1) * P, :])
                w_sb[name][kd] = w

            def load_xt(kd, c, eng):
                t = persist.tile([P, HC], bf16, tag=f"xt{kd}_{c}", name=f"xt{kd}_{c}")
                eng.dma_start(
                    t[:], xT[kd * P : (kd + 1) * P, c * HC : (c + 1) * HC]
                )
                xt_sb[kd][c] = t

            em_sb = []
            for kb in range(NSEQ):
                t = persist.tile([P, L], bf16, tag=f"em{kb}", name=f"em{kb}")
                em_sb.append(t)

            def emit_em(kb):
                nc.sync.dma_start(em_sb[kb][:], expmT[kb * P : (kb + 1) * P, :])

            # k00/q00-critical loads alternate between the two HWDGE queues
            # (SP and ACT) so their ~600ns serial issue costs overlap.
            for kd in range(0, KD, 2):
                load_xt(kd, 0, nc.sync)
                load_w("k", wk, kd, nc.sync)
            for kd in range(1, KD, 2):
                load_xt(kd, 0, nc.scalar)
                load_w("k", wk, kd, nc.scalar)
            for kd in range(KD):
                load_w("q", wq, kd, nc.scalar)
            for kd in range(KD):
                load_xt(kd, 1, nc.sync)
            for kb in range(4):
                emit_em(kb)
            for kd in range(0, KD, 2):
                load_w("v", wv, kd, nc.scalar)
            for kd in range(1, KD, 2):
                load_w("v", wv, kd, nc.sync)
            for kb in range(4, NSEQ):
                emit_em(kb)

            # KT/QT panels: [128 part = head-pair (2 heads x 64 hd), 512 seq]
            qt_sb = [
                [
                    persist.tile([P, QP], bf16, tag=f"qt{p}_{j}", name=f"qt{p}_{j}")
                    for j in range(NQP)
                ]
                for p in range(2)
            ]
            kt_sb = [
                [
                    persist.tile([P, QP], bf16, tag=f"kt{p}_{j}", name=f"kt{p}_{j}")
                    for j in range(NQP)
                ]
                for p in range(2)
            ]

            # V_all[:, kb*4+h, 0:64] = V block; [..., 64:128] = 1.0 (ones for
            # the softmax-denominator rows of the PV matmul). Ones memset on
            # GpSimd so DVE stays free during startup.
            v_all = persist.tile([P, NSEQ * HPC, P], bf16, tag="v_all")
            nc.gpsimd.memset(v_all[:, :, HD:P], 1.0)

            # PSUM->SBUF drain copies run on GpSimd (otherwise idle) so DVE
            # stays free for the per-iteration mask multiplies and PSUM banks
            # are released without queueing behind DVE work.
            def proj_qk(name, dest, p, j):
                c, co = divmod(j, 2)
                ps = ps_proj.tile([P, QP], f32, tag="proj", name="ps_proj")
                for kd in range(KD):
                    nc.tensor.matmul(
                        ps[:],
                        lhsT=w_sb[name][kd][:, p * P : (p + 1) * P],
                        rhs=xt_sb[kd][c][:, co * QP : (co + 1) * QP],
                        start=(kd == 0),
                        stop=(kd == KD - 1),
                    )
                nc.vector.tensor_copy(out=dest[p][j][:], in_=ps[:])

            def proj_v(kb):
                c, co = divmod(kb, NSEQ // 2)
                pv = ps_proj.tile([P, QP], f32, tag="proj", name="ps_projv")
                for kd in range(KD):
                    nc.tensor.matmul(
                        pv[:, :GW],
                        lhsT=xt_sb[kd][c][:, co * P : (co + 1) * P],
                        rhs=w_sb["v"][kd][:],
                        start=(kd == 0),
                        stop=(kd == KD - 1),
                    )
                nc.vector.tensor_copy(
                    out=v_all[:, kb * HPC : (kb + 1) * HPC, 0:HD],
                    in_=pv[:, :GW].rearrange("p (h d) -> p h d", h=HPC),
                )

            # ---- just-in-time projection schedule ----
            # Iteration index t = ((j*2 + hp)*16 + kb). Each projection task
            # is emitted a few iterations before the first attention matmul
            # that needs it, so the TensorE stream mixes projection and
            # attention work and never runs a long ACT-idle head phase.
            LEAD = 5
            QLEAD = 8
            tasks = []  # (emit_t, seq, fn)
            for kp in range(1, NQP):
                tasks.append((4 * kp - LEAD, lambda kp=kp: proj_qk("k", kt_sb, 0, kp)))
            tasks.append((16 - LEAD, lambda: proj_qk("k", kt_sb, 1, 0)))
            tasks.append((16 - LEAD + 1, lambda: proj_qk("q", qt_sb, 1, 0)))
            for kp in range(1, NQP):
                tasks.append(
                    (16 + 4 * kp - LEAD, lambda kp=kp: proj_qk("k", kt_sb, 1, kp))
                )
            for j in range(1, NQP):
                for hp in range(2):
                    tasks.append(
                        (
                            32 * j + 16 * hp - QLEAD,
                            lambda hp=hp, j=j: proj_qk("q", qt_sb, hp, j),
                        )
                    )
            for kb in range(NSEQ):
                tasks.append((max(0, kb - 1), lambda kb=kb: proj_v(kb)))
            tasks.sort(key=lambda x: x[0])
            task_i = 0

            # upfront: the two panels attention iteration 0 needs. Their
            # matmuls are interleaved at kd granularity: both chains are
            # paced by the same xt/w DMA arrivals, so interleaving finishes
            # both ~when the last input lands instead of serially.
            ps_k = ps_proj.tile([P, QP], f32, tag="proj", name="ps_k00")
            ps_q = ps_proj.tile([P, QP], f32, tag="proj", name="ps_q00")
            for kd in range(KD):
                for ps0, name in ((ps_k, "k"), (ps_q, "q")):
                    nc.tensor.matmul(
                        ps0[:],
                        lhsT=w_sb[name][kd][:, 0:P],
                        rhs=xt_sb[kd][0][:, 0:QP],
                        start=(kd == 0),
                        stop=(kd == KD - 1),
                    )
            nc.vector.tensor_copy(out=kt_sb[0][0][:], in_=ps_k[:])
            nc.vector.tensor_copy(out=qt_sb[0][0][:], in_=ps_q[:])

            # deferred normalize: the reciprocal waits on an SBUF-shift DMA,
            # and DVE executes in order — emitting the chain at the hp
            # boundary would stall the next sweep's multiplies behind it.
            # Instead it is emitted a few iterations into the next sweep.
            pending_norm = []

            def emit_norm():
                for fn in pending_norm:
                    fn()
                pending_norm.clear()

            # ---- attention ----
            for t in range(NITER):
                j, r = divmod(t, 2 * NSEQ)
                hp, kb = divmod(r, NSEQ)

                if kb == 0:
                    po = {
                        i: ps_o.tile([P, QP], f32, tag=f"o{i}", name=f"po{i}")
                        for i in range(2)
                    }

                kp, ko = divmod(kb, NSEQ // NQP)
                ps = ps_s.tile([P, 2 * QP], f32, tag="s")
                for i in range(2):
                    o = i * HD
                    nc.tensor.matmul(
                        ps[:, i * QP : (i + 1) * QP],
                        lhsT=kt_sb[hp][kp][o : o + HD, ko * P : (ko + 1) * P],
                        rhs=qt_sb[hp][j][o : o + HD, :],
                        start=True,
                        stop=True,
                        tile_position=(o, 0),
                    )
                e = e_pool.tile([P, 2 * QP], bf16, tag="e")
                nc.scalar.activation(e[:], ps[:], mybir.ActivationFunctionType.Exp)

                # JIT projections go after this iteration's scores so the
                # EXP stream is never delayed by projection matmuls
                while task_i < len(tasks) and tasks[task_i][0] <= t:
                    tasks[task_i][1]()
                    task_i += 1
                if kb == 2:
                    emit_norm()
                # one DVE multiply for both heads: mask tile broadcast along
                # a stride-0 middle dim
                eh = eh_pool.tile([P, 2 * QP], bf16, tag="eh")
                em_b = (
                    em_sb[kb][:, j * QP : (j + 1) * QP]
                    .unsqueeze(1)
                    .broadcast_to([P, 2, QP])
                )
                nc.vector.tensor_tensor(
                    eh[:].rearrange("p (a b) -> p a b", a=2),
                    e[:].rearrange("p (a b) -> p a b", a=2),
                    em_b,
                    mybir.AluOpType.mult,
                )
                for i in range(2):
                    h = 2 * hp + i
                    nc.tensor.matmul(
                        po[i][:],
                        lhsT=v_all[:, kb * HPC + h, :],
                        rhs=eh[:, i * QP : (i + 1) * QP],
                        start=(kb == 0),
                        stop=(kb == NSEQ - 1),
                    )

                if kb == NSEQ - 1:
                    # drain both heads' PSUM promptly so the next head-pair's
                    # PV accumulation can claim the banks; kick off the
                    # denominator-shift DMA now, defer the DVE chain. Both
                    # heads share one [128, 1024] osb so the shift, the
                    # reciprocal, the multiply, and the store are one
                    # instruction each per sweep.
                    osb = osb_pool.tile([P, 2 * QP], f32, tag="osb", name="osb")
                    for i in range(2):
                        nc.vector.tensor_copy(
                            osb[:, i * QP : (i + 1) * QP], po[i][:]
                        )
                    # operands of DVE ops must share a partition base, so
                    # shift the denominator rows down via an SBUF->SBUF DMA
                    r_t = osb_pool.tile([HD, 2 * QP], f32, tag="r_t", name="r_t")
                    nc.sync.dma_start(r_t[:], osb[HD : 2 * HD, :])

                    def norm(hp=hp, j=j, osb=osb, r_t=r_t):
                        rc = osb_pool.tile([HD, 2 * QP], f32, tag="rc", name="rc")
                        nc.vector.reciprocal_approx_fast(out=rc[:], in_=r_t[:])
                        res = res_pool.tile([HD, 2 * QP], f32, tag="res", name="res")
                        nc.vector.tensor_tensor(
                            res[:], osb[0:HD, :], rc[:], mybir.AluOpType.mult
                        )
                        # res cols [head 2hp | head 2hp+1] -> outT row blocks;
                        # SBUF src keeps its partition dim outermost, the DRAM
                        # dst AP is permuted to match the iteration order
                        nc.sync.dma_start(
                            outT[
                                2 * hp * HD : (2 * hp + 2) * HD,
                                j * QP : (j + 1) * QP,
                            ].rearrange("(a p) b -> p a b", a=2),
                            res[:].rearrange("p (a b) -> p a b", a=2),
                        )

                    pending_norm.append(norm)
            emit_norm()

    nc.compile()
    return nc


def _prep_in_maps(x, attention_mask, Wq, Wk, Wv):
    x = np.asarray(x, np.float32)
    attention_mask = np.asarray(attention_mask, np.float32)
    Wq = np.asarray(Wq, np.float32)
    Wk = np.asarray(Wk, np.float32)
    Wv = np.asarray(Wv, np.float32)

    xT_b = [np.ascontiguousarray(x[b].T).astype(BF16) for b in range(B)]
    expmT_b = [
        np.exp(MASK_SCALE * attention_mask[b].T, dtype=np.float32).astype(BF16)
        for b in range(B)
    ]
    in_maps = []
    for c in range(N_CORES):
        b, hg = divmod(c, HPC)
        sl = slice(hg * GW, (hg + 1) * GW)
        in_maps.append(
            {
                "xT": xT_b[b],
                "expmT": expmT_b[b],
                "wq": np.ascontiguousarray(Wq[:, sl] * SCALE).astype(BF16),
                "wk": np.ascontiguousarray(Wk[:, sl]).astype(BF16),
                "wv": np.ascontiguousarray(Wv[:, sl]).astype(BF16),
            }
        )
    return in_maps


def kernel(x, attention_mask, Wq, bq, Wk, bk, Wv, bv, **_unused):
    # bq/bk/bv are zeros per the problem spec and are not applied.
    if "nc" not in _CACHE:
        _CACHE["nc"] = _build_nc()
    nc = _CACHE["nc"]

    in_maps = _prep_in_maps(x, attention_mask, Wq, Wk, Wv)
    r = run_bass_kernel_spmd(nc, in_maps, core_ids=list(range(N_CORES)))
    _CACHE["last_results"] = r

    out = np.empty((B, L, DIM), np.float32)
    for c in range(N_CORES):
        b, hg = divmod(c, HPC)
        out[b, :, hg * GW : (hg + 1) * GW] = r.results[c]["outT"].T
    return out


# revision 28
# speedup vs baseline: 1.0355x; 1.0355x over previous
"""Multi-head attention (B=2, L=2048, DIM=1024, H=16) on 8 TRN2 NeuronCores.

Sharding: core c = (batch b = c//4, head-group hg = c%4 of 4 heads / 256 dims).
Data parallel over B, tensor parallel over heads; Q/K/V weights column-sharded.
Each core is fully independent (no collectives); host gathers the 8 output
shards.

Per-core layout trick: everything is computed transposed (seq on the free
axis) so no on-device transposes are needed:
  QT/KT [hd, seq]  <- matmul(lhsT=W_slice, rhs=xT)       (xT transposed on host)
  ST    [k, q]     <- matmul(lhsT=KT_head, rhs=QT_head)  (= scores transposed)
  E     = exp(ST)         (max-subtraction skipped: logits are N(0,1)-scaled,
                           mask only subtracts -> exp stays in [e^-65, e^5])
  Emask = E * exp(-60*mask)^T                            (mask exp'd on host)
  OT    [hd+ones, q] <- matmul(lhsT=[V | ones], rhs=Emask) accumulated over k;
                        rows 64..127 give the softmax denominator replicated,
                        so out = OT[0:64] / OT[64:128] needs no partition
                        broadcast.
The 1/sqrt(64) score scale is folded into Wq on the host.
Biases are zeros per the problem spec and are skipped.

Pipeline structure (v2): the kernel is ACT(exp)-throughput-bound, so the
instruction streams are laid out to keep ScalarE fed from ~first-DMA-landing
to the end and to keep TensorE saturated (its DVFS p-state only reaches
2.4 GHz when continuously busy):
  - All QK/V projections are emitted just-in-time, interleaved into the
    attention iteration stream instead of as a serial head phase.
  - The mask multiply is one [128,1024] tensor_tensor per (head-pair, kb)
    using a stride-0 broadcast AP on the [128,512] mask tile (covers both
    heads in one DVE op).
  - The softmax reciprocal runs on DVE (reciprocal_approx_fast, ~51 ULP)
    instead of Ln+Exp on ScalarE.
  - The PV ones-rows are memset on GpSimd (free engine at startup).
"""

import sys

for _p in ("/opt/trn_rl_repo",):
    if _p not in sys.path:
        sys.path.append(_p)

import numpy as np
import ml_dtypes

import concourse.tile as tile
from concourse import bacc, mybir
from concourse.bass_utils import run_bass_kernel_spmd

BF16 = ml_dtypes.bfloat16

B, L, DIM, H = 2, 2048, 1024, 16
HPC = 4          # heads per core
HD = DIM // H    # 64
GW = HPC * HD    # 256, head-group width per core
N_CORES = 8
MASK_SCALE = -60.0
SCALE = float(HD) ** -0.5

P = 128
KD = DIM // P        # 8  contraction blocks for projections
NSEQ = L // P        # 16 seq blocks (k blocks)
QP = 512             # q panel width
NQP = L // QP        # 4 q panels
NITER = NQP * 2 * NSEQ  # 128 attention iterations (j, hp, kb)

_CACHE = {}


def _build_nc():
    f32 = mybir.dt.float32
    bf16 = mybir.dt.bfloat16

    nc = bacc.Bacc("TRN2", target_bir_lowering=False)

    xT = nc.declare_dram_parameter("xT", [DIM, L], bf16, isOutput=False)
    expmT = nc.declare_dram_parameter("expmT", [L, L], bf16, isOutput=False)
    wq = nc.declare_dram_parameter("wq", [DIM, GW], bf16, isOutput=False)
    wk = nc.declare_dram_parameter("wk", [DIM, GW], bf16, isOutput=False)
    wv = nc.declare_dram_parameter("wv", [DIM, GW], bf16, isOutput=False)
    outT = nc.declare_dram_parameter("outT", [GW, L], f32, isOutput=True)

    with tile.TileContext(nc) as tc:
        with (
            tc.tile_pool(name="persist", bufs=1) as persist,
            tc.tile_pool(name="e", bufs=6) as e_pool,
            tc.tile_pool(name="eh", bufs=6) as eh_pool,
            tc.tile_pool(name="osb", bufs=2) as osb_pool,
            tc.tile_pool(name="res", bufs=2) as res_pool,
            tc.tile_pool(name="ps_proj", bufs=2, space="PSUM") as ps_proj,
            tc.tile_pool(name="ps_s", bufs=2, space="PSUM") as ps_s,
            tc.tile_pool(name="ps_o", bufs=1, space="PSUM") as ps_o,
        ):
            # ---- input DMA ----
            # Every dma_start costs ~600ns of serial issue time on its DGE
            # queue (SP), so the DMA plan minimizes instruction count and
            # splits issue across the two HWDGE engines (SP + ACT):
            #   SP:  xt halves 0 + wk (the k00 deps), mask rows 0-3,
            #        xt halves 1, mask rows 4-15
            #   ACT: wq, wv (ACT is idle until the first EXP anyway)
            # The mask is loaded once as 16 persistent whole-row tiles and
            # stays resident for all four q panels.
            HC = L // 2  # xt column-half width
            xt_sb = [[None] * 2 for _ in range(KD)]
            w_sb = {"q": [None] * KD, "k": [None] * KD, "v": [None] * KD}

            def load_w(name, dram, kd, eng):
                w = persist.tile([P, GW], bf16, tag=f"w{name}{kd}", name=f"w{name}{kd}")
                eng.dma_start(w[:], dram[kd * P : (kd + 1) * P, :])
                w_sb[name][kd] = w

            def load_xt(kd, c, eng):
                t = persist.tile([P, HC], bf16, tag=f"xt{kd}_{c}", name=f"xt{kd}_{c}")
                eng.dma_start(
                    t[:], xT[kd * P : (kd + 1) * P, c * HC : (c + 1) * HC]
                )
                xt_sb[kd][c] = t

            em_sb = []
            for kb in range(NSEQ):
                t = persist.tile([P, L], bf16, tag=f"em{kb}", name=f"em{kb}")
                em_sb.append(t)

            def emit_em(kb):
                nc.sync.dma_start(em_sb[kb][:], expmT[kb * P : (kb + 1) * P, :])

            # Inputs: SP issues the k00-critical xt/wk stream; the second
            # HWDGE queue (ACT) issues wq/wv during its pre-EXP idle. (Finer
            # cross-queue splits of the xt/wk stream were measured slower.)
            for kd in range(KD):
                load_xt(kd, 0, nc.sync)
                load_w("k", wk, kd, nc.sync)
            for kd in range(KD):
                load_w("q", wq, kd, nc.scalar)
            for kd in range(KD):
                load_w("v", wv, kd, nc.scalar)
            for kb in range(4):
                emit_em(kb)
            for kd in range(KD):
                load_xt(kd, 1, nc.sync)
            for kb in range(4, NSEQ):
                emit_em(kb)

            # KT/QT panels: [128 part = head-pair (2 heads x 64 hd), 512 seq]
            qt_sb = [
                [
                    persist.tile([P, QP], bf16, tag=f"qt{p}_{j}", name=f"qt{p}_{j}")
                    for j in range(NQP)
                ]
                for p in range(2)
            ]
            kt_sb = [
                [
                    persist.tile([P, QP], bf16, tag=f"kt{p}_{j}", name=f"kt{p}_{j}")
                    for j in range(NQP)
                ]
                for p in range(2)
            ]

            # V_all[:, kb*4+h, 0:64] = V block; [..., 64:128] = 1.0 (ones for
            # the softmax-denominator rows of the PV matmul). Ones memset on
            # GpSimd so DVE stays free during startup.
            v_all = persist.tile([P, NSEQ * HPC, P], bf16, tag="v_all")
            nc.gpsimd.memset(v_all[:, :, HD:P], 1.0)

            # PSUM->SBUF drain copies run on GpSimd (otherwise idle) so DVE
            # stays free for the per-iteration mask multiplies and PSUM banks
            # are released without queueing behind DVE work.
            def proj_qk(name, dest, p, j):
                c, co = divmod(j, 2)
                ps = ps_proj.tile([P, QP], f32, tag="proj", name="ps_proj")
                for kd in range(KD):
                    nc.tensor.matmul(
                        ps[:],
                        lhsT=w_sb[name][kd][:, p * P : (p + 1) * P],
                        rhs=xt_sb[kd][c][:, co * QP : (co + 1) * QP],
                        start=(kd == 0),
                        stop=(kd == KD - 1),
                    )
                nc.vector.tensor_copy(out=dest[p][j][:], in_=ps[:])

            def proj_v(kb):
                c, co = divmod(kb, NSEQ // 2)
                pv = ps_proj.tile([P, QP], f32, tag="proj", name="ps_projv")
                for kd in range(KD):
                    nc.tensor.matmul(
                        pv[:, :GW],
                        lhsT=xt_sb[kd][c][:, co * P : (co + 1) * P],
                        rhs=w_sb["v"][kd][:],
                        start=(kd == 0),
                        stop=(kd == KD - 1),
                    )
                nc.vector.tensor_copy(
                    out=v_all[:, kb * HPC : (kb + 1) * HPC, 0:HD],
                    in_=pv[:, :GW].rearrange("p (h d) -> p h d", h=HPC),
                )

            # ---- just-in-time projection schedule ----
            # Iteration index t = ((j*2 + hp)*16 + kb). Each projection task
            # is emitted a few iterations before the first attention matmul
            # that needs it, so the TensorE stream mixes projection and
            # attention work and never runs a long ACT-idle head phase.
            LEAD = 5
            QLEAD = 8
            tasks = []  # (emit_t, seq, fn)
            for kp in range(1, NQP):
                tasks.append((4 * kp - LEAD, lambda kp=kp: proj_qk("k", kt_sb, 0, kp)))
            tasks.append((16 - LEAD, lambda: proj_qk("k", kt_sb, 1, 0)))
            tasks.append((16 - LEAD + 1, lambda: proj_qk("q", qt_sb, 1, 0)))
            for kp in range(1, NQP):
                tasks.append(
                    (16 + 4 * kp - LEAD, lambda kp=kp: proj_qk("k", kt_sb, 1, kp))
                )
            for j in range(1, NQP):
                for hp in range(2):
                    tasks.append(
                        (
                            32 * j + 16 * hp - QLEAD,
                            lambda hp=hp, j=j: proj_qk("q", qt_sb, hp, j),
                        )
                    )
            for kb in range(NSEQ):
                tasks.append((max(0, kb - 1), lambda kb=kb: proj_v(kb)))
            tasks.sort(key=lambda x: x[0])
            task_i = 0

            # upfront: the two panels attention iteration 0 needs. Their
            # matmuls are interleaved at kd granularity: both chains are
            # paced by the same xt/w DMA arrivals, so interleaving finishes
            # both ~when the last input lands instead of serially.
            ps_k = ps_proj.tile([P, QP], f32, tag="proj", name="ps_k00")
            ps_q = ps_proj.tile([P, QP], f32, tag="proj", name="ps_q00")
            for kd in range(KD):
                for ps0, name in ((ps_k, "k"), (ps_q, "q")):
                    nc.tensor.matmul(
                        ps0[:],
                        lhsT=w_sb[name][kd][:, 0:P],
                        rhs=xt_sb[kd][0][:, 0:QP],
                        start=(kd == 0),
                        stop=(kd == KD - 1),
                    )
            nc.vector.tensor_copy(out=kt_sb[0][0][:], in_=ps_k[:])
            nc.vector.tensor_copy(out=qt_sb[0][0][:], in_=ps_q[:])

            # deferred normalize: the reciprocal waits on an SBUF-shift DMA,
            # and DVE executes in order — emitting the chain at the hp
            # boundary would stall the next sweep's multiplies behind it.
            # Instead it is emitted a few iterations into the next sweep.
            pending_norm = []

            def emit_norm():
                for fn in pending_norm:
                    fn()
                pending_norm.clear()

            # ---- attention ----
            for t in range(NITER):
                j, r = divmod(t, 2 * NSEQ)
                hp, kb = divmod(r, NSEQ)

                if kb == 0:
                    po = {
                        i: ps_o.tile([P, QP], f32, tag=f"o{i}", name=f"po{i}")
                        for i in range(2)
                    }

                kp, ko = divmod(kb, NSEQ // NQP)
                ps = ps_s.tile([P, 2 * QP], f32, tag="s")
                for i in range(2):
                    o = i * HD
                    nc.tensor.matmul(
                        ps[:, i * QP : (i + 1) * QP],
                        lhsT=kt_sb[hp][kp][o : o + HD, ko * P : (ko + 1) * P],
                        rhs=qt_sb[hp][j][o : o + HD, :],
                        start=True,
                        stop=True,
                        tile_position=(o, 0),
                    )
                e = e_pool.tile([P, 2 * QP], bf16, tag="e")
                nc.scalar.activation(e[:], ps[:], mybir.ActivationFunctionType.Exp)

                # JIT projections go after this iteration's scores so the
                # EXP stream is never delayed by projection matmuls
                while task_i < len(tasks) and tasks[task_i][0] <= t:
                    tasks[task_i][1]()
                    task_i += 1
                if kb == 2:
                    emit_norm()
                # one DVE multiply for both heads: mask tile broadcast along
                # a stride-0 middle dim
                eh = eh_pool.tile([P, 2 * QP], bf16, tag="eh")
                em_b = (
                    em_sb[kb][:, j * QP : (j + 1) * QP]
                    .unsqueeze(1)
                    .broadcast_to([P, 2, QP])
                )
                nc.vector.tensor_tensor(
                    eh[:].rearrange("p (a b) -> p a b", a=2),
                    e[:].rearrange("p (a b) -> p a b", a=2),
                    em_b,
                    mybir.AluOpType.mult,
                )
                for i in range(2):
                    h = 2 * hp + i
                    nc.tensor.matmul(
                        po[i][:],
                        lhsT=v_all[:, kb * HPC + h, :],
                        rhs=eh[:, i * QP : (i + 1) * QP],
                        start=(kb == 0),
                        stop=(kb == NSEQ - 1),
                    )

                if kb == NSEQ - 1:
                    # drain both heads' PSUM promptly so the next head-pair's
                    # PV accumulation can claim the banks; kick off the
                    # denominator-shift DMA now, defer the DVE chain. Both
                    # heads share one [128, 1024] osb so the shift, the
                    # reciprocal, the multiply, and the store are one
                    # instruction each per sweep.
                    osb = osb_pool.tile([P, 2 * QP], f32, tag="osb", name="osb")
                    for i in range(2):
                        nc.vector.tensor_copy(
                            osb[:, i * QP : (i + 1) * QP], po[i][:]
                        )
                    # operands of DVE ops must share a partition base, so
                    # shift the denominator rows down via an SBUF->SBUF DMA
                    r_t = osb_pool.tile([HD, 2 * QP], f32, tag="r_t", name="r_t")
                    nc.sync.dma_start(r_t[:], osb[HD : 2 * HD, :])

                    def norm(hp=hp, j=j, osb=osb, r_t=r_t):
                        rc = osb_pool.tile([HD, 2 * QP], f32, tag="rc", name="rc")
                        nc.vector.reciprocal_approx_fast(out=rc[:], in_=r_t[:])
                        res = res_pool.tile([HD, 2 * QP], f32, tag="res", name="res")
                        nc.vector.tensor_tensor(
                            res[:], osb[0:HD, :], rc[:], mybir.AluOpType.mult
                        )
                        # res cols [head 2hp | head 2hp+1] -> outT row blocks;
                        # SBUF src keeps its partition dim outermost, the DRAM
                        # dst AP is permuted to match the iteration order
                        nc.sync.dma_start(
                            outT[
                                2 * hp * HD : (2 * hp + 2) * HD,
                                j * QP : (j + 1) * QP,
                            ].rearrange("(a p) b -> p a b", a=2),
                            res[:].rearrange("p (a b) -> p a b", a=2),
                        )

                    pending_norm.append(norm)
            emit_norm()

    nc.compile()
    return nc


def _prep_in_maps(x, attention_mask, Wq, Wk, Wv):
    x = np.asarray(x, np.float32)
    attention_mask = np.asarray(attention_mask, np.float32)
    Wq = np.asarray(Wq, np.float32)
    Wk = np.asarray(Wk, np.float32)
    Wv = np.asarray(Wv, np.float32)

    xT_b = [np.ascontiguousarray(x[b].T).astype(BF16) for b in range(B)]
    expmT_b = [
        np.exp(MASK_SCALE * attention_mask[b].T, dtype=np.float32).astype(BF16)
        for b in range(B)
    ]
    in_maps = []
    for c in range(N_CORES):
        b, hg = divmod(c, HPC)
        sl = slice(hg * GW, (hg + 1) * GW)
        in_maps.append(
            {
                "xT": xT_b[b],
                "expmT": expmT_b[b],
                "wq": np.ascontiguousarray(Wq[:, sl] * SCALE).astype(BF16),
                "wk": np.ascontiguousarray(Wk[:, sl]).astype(BF16),
                "wv": np.ascontiguousarray(Wv[:, sl]).astype(BF16),
            }
        )
    return in_maps


def kernel(x, attention_mask, Wq, bq, Wk, bk, Wv, bv, **_unused):
    # bq/bk/bv are zeros per the problem spec and are not applied.
    if "nc" not in _CACHE:
        _CACHE["nc"] = _build_nc()
    nc = _CACHE["nc"]

    in_maps = _prep_in_maps(x, attention_mask, Wq, Wk, Wv)
    r = run_bass_kernel_spmd(nc, in_maps, core_ids=list(range(N_CORES)))
    _CACHE["last_results"] = r

    out = np.empty((B, L, DIM), np.float32)
    for c in range(N_CORES):
        b, hg = divmod(c, HPC)
        out[b, :, hg * GW : (hg + 1) * GW] = r.results[c]["outT"].T
    return out
